# revision 16
# baseline (speedup 1.0000x reference)
"""LNN / echo-state step on 8 TRN2 NeuronCores.

Computes state = 0.7*prev_state + 0.3*tanh(inputs @ Wi^T + prev_state @ Wr^T)
for B=8192, IN=2048, R=4096 (fp32 in/out).

Strategy: data-parallel over batch. Each of the 8 cores gets a 1024-row batch
shard and the full (replicated) weights, computes its shard's output with no
collectives, and the host reassembles.

Per-core kernel (fp8e4m3 DoubleRow matmuls — 2 MACs/cell/cycle, ~1.7x over
fp32r/bf16 streaming):
  - Host quantizes activations (x, h) and weights to fp8 e4m3. Weights are
    pre-scaled by 64 so their std (~0.02) lands in e4m3's normal range;
    the 1/64 is folded into the tanh via the scalar engine's scale arg.
  - out^T[r, b] accumulates over 24 DoubleRow pairs (256 contraction rows
    each): pairs 0-7 contract x^T against Wi^T, pairs 8-23 contract h^T
    against Wr^T. fp8 activations (6 MB) stay resident in SBUF; fp8 weight
    pair-tiles stream from HBM per output m-tile.
  - epilogue per [128, 512] tile: tanh(psum/64) on ScalarE, then
    out = 0.7*h_fp32 + 0.3*tanh on VectorE (h^T fp32 tiles streamed from
    HBM per m-tile), DMA back to HBM.

Host-side numpy does the transposes/tiling/quantization so every DMA is
contiguous.
"""

import numpy as np
import ml_dtypes

import concourse.bass as bass
import concourse.mybir as mybir
from concourse import bacc
from concourse.tile import TileContext

P = 128
B_FULL, IN_DIM, R_DIM = 8192, 2048, 4096
N_CORES = 8
B_SHARD = B_FULL // N_CORES
LEAK = 0.3
W_SCALE = 64.0
USE_SW = True  # DoubleRowSwInterleave (contiguous weight loads) vs DoubleRow

F8NP = ml_dtypes.float8_e4m3


import contextlib


@contextlib.contextmanager
def _null_ctx():
    yield None


def build_program(in_dim=IN_DIM, r_dim=R_DIM, b_shard=B_SHARD, kpc=6, n_tile=512,
                  t_loop=None, probe_same_w=False, acts_once=False):
    """Emit the per-core Bass program. Returns (nc, meta).

    t_loop: if set, wrap the whole body in a hardware For_i loop that runs it
    t_loop times back-to-back on device (timing use only — slope over t_loop
    cancels dispatch overhead exactly).
    probe_same_w / acts_once: timing-probe variants (wrong results)."""
    kp_x = in_dim // (2 * P)    # DoubleRow pairs from the input matmul
    kp_h = r_dim // (2 * P)     # DoubleRow pairs from the reservoir matmul
    kp = kp_x + kp_h            # total fused contraction pairs
    mt = r_dim // P             # output row tiles (R on partitions)
    nt = b_shard // n_tile      # output column tiles
    nchunk = kp // kpc          # weight DMA chunks per m-tile
    assert kp % kpc == 0 and b_shard % n_tile == 0

    f32 = mybir.dt.float32
    f8 = mybir.dt.float8e4
    Tanh = mybir.ActivationFunctionType.Tanh
    DR = (mybir.MatmulPerfMode.DoubleRowSwInterleave if USE_SW
          else mybir.MatmulPerfMode.DoubleRow)

    nc = bacc.Bacc("TRN2", target_bir_lowering=False, debug=False)

    acts_d = nc.dram_tensor("acts", [kp, P, 2 * b_shard], f8, kind="ExternalInput")
    wts_d = nc.dram_tensor("wts", [mt, nchunk, P, kpc * 2 * P], f8, kind="ExternalInput")
    h32_d = nc.dram_tensor("h32", [mt, P, b_shard], f32, kind="ExternalInput")
    out_d = nc.dram_tensor("out", [mt, P, b_shard], f32, kind="ExternalOutput")

    with TileContext(nc) as tc:
        with (
            tc.tile_pool(name="act_pool", bufs=kp) as apool,
            tc.tile_pool(name="w_pool", bufs=10) as wpool,
            tc.tile_pool(name="h_pool", bufs=3) as hpool,
            tc.tile_pool(name="t_pool", bufs=2) as tpool,
            tc.tile_pool(name="o_pool", bufs=2) as opool,
            tc.tile_pool(name="ps_pool", bufs=4, space="PSUM") as pspool,
        ):
            act_tiles = []

            def emit_acts():
                for k in range(kp):
                    at = apool.tile([P, 2, b_shard], f8, tag="act", name=f"act{k}")
                    nc.sync.dma_start(at[:], acts_d[k])
                    act_tiles.append(at)

            if acts_once and t_loop is not None:
                emit_acts()
            with (tc.For_i(0, t_loop) if t_loop is not None else _null_ctx()):
                if not (acts_once and t_loop is not None):
                    emit_acts()
                for m in range(mt):
                    ht = hpool.tile([P, b_shard], f32, tag="h")
                    nc.sync.dma_start(ht[:], h32_d[m])
                    psums = [pspool.tile([P, n_tile], f32, tag="ps", name=f"ps{m}_{n}")
                             for n in range(nt)]
                    wcs = []
                    for ch in range(nchunk):
                        wshape = [P, kpc, 2 * P] if USE_SW else [P, kpc, 2, P]
                        wc = wpool.tile(wshape, f8, tag="w")
                        nc.sync.dma_start(wc[:], wts_d[m, ch])
                        wcs.append(wc)
                    # all k-accumulation for one PSUM bank back-to-back (avoids
                    # per-MM PSUM bank cycling)
                    for n in range(nt):
                        for ch in range(nchunk):
                            for kl in range(kpc):
                                k = ch * kpc + kl
                                if probe_same_w:
                                    lhsT = wcs[0][:, 0]
                                else:
                                    lhsT = wcs[ch][:, kl]
                                rhs = act_tiles[k][:, :, n * n_tile:(n + 1) * n_tile]
                                nc.tensor.matmul(
                                    psums[n][:],
                                    lhsT,
                                    rhs,
                                    start=(k == 0),
                                    stop=(k == kp - 1),
                                    perf_mode=DR,
                                )
                    for n in range(nt):
                        t = tpool.tile([P, n_tile], f32, tag="t")
                        nc.scalar.activation(t[:], psums[n][:], Tanh, scale=1.0 / W_SCALE)
                        o = opool.tile([P, n_tile], f32, tag="o")
                        h_slice = ht[:, n * n_tile:(n + 1) * n_tile]
                        nc.vector.tensor_scalar_mul(o[:], h_slice, 1.0 - LEAK)
                        nc.vector.scalar_tensor_tensor(
                            o[:], t[:], LEAK, o[:],
                            mybir.AluOpType.mult, mybir.AluOpType.add,
                        )
                        nc.sync.dma_start(out_d[m, :, n * n_tile:(n + 1) * n_tile], o[:])

    nc.compile()
    meta = dict(in_dim=in_dim, r_dim=r_dim, b_shard=b_shard, kpc=kpc,
                n_tile=n_tile, kp_x=kp_x, kp_h=kp_h, kp=kp, mt=mt, nt=nt,
                nchunk=nchunk)
    return nc, meta


def pack_weights(input_weights, reservoir_weights, kpc=6):
    """[R, IN] + [R, R] fp32 -> [mt, nchunk, P, kpc*2*P] fp8, tiled for
    contiguous DMA and DoubleRow pair layout [P, 2, P] per (m, pair)."""
    w = np.concatenate(
        [np.ascontiguousarray(input_weights.T), np.ascontiguousarray(reservoir_weights.T)],
        axis=0,
    )  # [in+r, r]: w[k, r]
    w8 = (w * W_SCALE).astype(F8NP)
    k_dim, r_dim = w8.shape
    kp, mt = k_dim // (2 * P), r_dim // P
    nchunk = kp // kpc
    # w8[k, r]: k = 256*pair + 128*j + p, r = 128*m + c -> [m, pair, p, j, c]
    t = w8.reshape(kp, 2, P, mt, P).transpose(3, 0, 2, 1, 4)  # [mt, kp, P, 2, P]
    if USE_SW:
        # SwInterleave layout per partition: A127 B127 A126 B126 ... A0 B0
        # (planes interleaved per column, columns reversed)
        t = np.ascontiguousarray(t[..., ::-1].transpose(0, 1, 2, 4, 3))
    t = t.reshape(mt, nchunk, kpc, P, 2, P).transpose(0, 1, 3, 2, 4, 5)
    return np.ascontiguousarray(t.reshape(mt, nchunk, P, kpc * 2 * P))


def pack_acts(x_shard, h_shard):
    """[b, in] + [b, r] fp32 -> [kp, P, 2*b] fp8 (transposed, DoubleRow
    pair-tiled: plane j at partition p covers k = 256*pair + 128*j + p)."""
    a = np.concatenate([x_shard.T, h_shard.T], axis=0)  # [in+r, b]
    a8 = a.astype(F8NP)
    k_dim, b = a8.shape
    return np.ascontiguousarray(a8.reshape(k_dim // (2 * P), 2 * P, b)
                                .reshape(-1, 2, P, b).transpose(0, 2, 1, 3)
                                .reshape(-1, P, 2 * b))


def pack_h32(h_shard):
    """[b, r] fp32 -> [mt, P, b] transposed tiles for the epilogue."""
    b, r = h_shard.shape
    return np.ascontiguousarray(h_shard.T.reshape(r // P, P, b))


_CACHE = {}


def make_in_maps(inputs, prev_state, input_weights, reservoir_weights):
    x = np.ascontiguousarray(np.asarray(inputs, dtype=np.float32))
    h = np.ascontiguousarray(np.asarray(prev_state, dtype=np.float32))
    wi = np.asarray(input_weights, dtype=np.float32)
    wr = np.asarray(reservoir_weights, dtype=np.float32)
    assert x.shape == (B_FULL, IN_DIM) and h.shape == (B_FULL, R_DIM)

    wts = pack_weights(wi, wr)
    in_maps = []
    for c in range(N_CORES):
        sl = slice(c * B_SHARD, (c + 1) * B_SHARD)
        in_maps.append({
            "acts": pack_acts(x[sl], h[sl]),
            "wts": wts,
            "h32": pack_h32(h[sl]),
        })
    return in_maps


def kernel(inputs, prev_state, input_weights, reservoir_weights):
    from concourse import bass_utils

    if "nc" not in _CACHE:
        _CACHE["nc"], _CACHE["meta"] = build_program()
    nc = _CACHE["nc"]

    in_maps = make_in_maps(inputs, prev_state, input_weights, reservoir_weights)
    res = bass_utils.run_bass_kernel_spmd(nc, in_maps, core_ids=list(range(N_CORES)))

    out = np.empty((B_FULL, R_DIM), dtype=np.float32)
    for c in range(N_CORES):
        o = res.results[c]["out"]  # [mt, P, b_shard]
        out[c * B_SHARD:(c + 1) * B_SHARD] = o.reshape(R_DIM, B_SHARD).T
    return out


# revision 20
# speedup vs baseline: 1.0140x; 1.0140x over previous
"""LNN / echo-state step on 8 TRN2 NeuronCores.

Computes state = 0.7*prev_state + 0.3*tanh(inputs @ Wi^T + prev_state @ Wr^T)
for B=8192, IN=2048, R=4096 (fp32 in/out).

Strategy: data-parallel over batch. Each of the 8 cores gets a 1024-row batch
shard and the full (replicated) weights, computes its shard's output with no
collectives, and the host reassembles.

Per-core kernel (fp8e4m3 DoubleRow matmuls — 2 MACs/cell/cycle, ~2x over
fp32r/bf16 streaming):
  - Host quantizes activations (x, h) and weights to fp8 e4m3. Weights are
    pre-scaled by 64 so their std (~0.02) lands in e4m3's normal range;
    the 1/64 is folded into the tanh via the scalar engine's scale arg.
  - out^T[r, b] accumulates over 24 DoubleRow pairs (256 contraction rows
    each): pairs 0-7 contract x^T against Wi^T, pairs 8-23 contract h^T
    against Wr^T. fp8 activations stay resident in SBUF; fp8 weights stream
    from HBM per output m-tile.
  - epilogue per [128, 512] tile: tanh(psum/64) on ScalarE, then
    out = 0.7*h + 0.3*tanh on VectorE. h arrives and out leaves as bf16
    (host up-casts the output) to halve epilogue HBM traffic.
  - DMA batching (DMA-stall was the measured bottleneck): acts load as one
    6.3 MB transfer, weights as 1.6 MB transfers (2 m-tiles each), h/out as
    1 MB transfers (4 m-tiles each). Mid-size (~0.3 MB) strided transfers
    only reach ~60% of HBM bandwidth; >=1 MB reaches ~80-97%.

Host-side numpy does the transposes/tiling/quantization so every DMA is
contiguous.
"""

import contextlib

import numpy as np
import ml_dtypes

import concourse.mybir as mybir
from concourse import bacc
from concourse.tile import TileContext

P = 128
B_FULL, IN_DIM, R_DIM = 8192, 2048, 4096
N_CORES = 8
B_SHARD = B_FULL // N_CORES
LEAK = 0.3
W_SCALE = 64.0
MG_W = 2   # m-tiles per weight DMA
MG_H = 4   # m-tiles per h/out DMA

F8NP = ml_dtypes.float8_e4m3
BF16NP = ml_dtypes.bfloat16


@contextlib.contextmanager
def _null_ctx():
    yield None


def build_program(in_dim=IN_DIM, r_dim=R_DIM, b_shard=B_SHARD, n_tile=512,
                  t_loop=None, no_in_dma=False, no_epilogue=False):
    """Emit the per-core Bass program. Returns (nc, meta).

    t_loop: if set, wrap the whole body in a hardware For_i loop that runs it
    t_loop times back-to-back on device (timing use only — slope over t_loop
    cancels dispatch overhead exactly).
    no_in_dma / no_epilogue: timing-probe variants (wrong results)."""
    kp = (in_dim + r_dim) // (2 * P)   # fused DoubleRow contraction pairs
    mt = r_dim // P                    # output row tiles (R on partitions)
    nt = b_shard // n_tile             # output column tiles
    assert b_shard % n_tile == 0 and mt % MG_W == 0 and mt % MG_H == 0

    f32 = mybir.dt.float32
    bf16 = mybir.dt.bfloat16
    f8 = mybir.dt.float8e4
    Tanh = mybir.ActivationFunctionType.Tanh
    DR = mybir.MatmulPerfMode.DoubleRowSwInterleave

    nc = bacc.Bacc("TRN2", target_bir_lowering=False, debug=False)

    acts_d = nc.dram_tensor("acts", [P, kp, 2, b_shard], f8, kind="ExternalInput")
    wts_d = nc.dram_tensor("wts", [mt // MG_W, P, MG_W, kp, 2 * P], f8,
                           kind="ExternalInput")
    h16_d = nc.dram_tensor("h16", [mt // MG_H, P, MG_H, b_shard], bf16,
                           kind="ExternalInput")
    out_d = nc.dram_tensor("out", [mt // MG_H, P, MG_H, b_shard], bf16,
                           kind="ExternalOutput")

    with TileContext(nc) as tc:
        with (
            tc.tile_pool(name="act_pool", bufs=1) as apool,
            tc.tile_pool(name="w_pool", bufs=3) as wpool,
            tc.tile_pool(name="h_pool", bufs=2) as hpool,
            tc.tile_pool(name="t_pool", bufs=2) as tpool,
            tc.tile_pool(name="o_pool", bufs=2) as opool,
            tc.tile_pool(name="ps_pool", bufs=4, space="PSUM") as pspool,
        ):
            at = None

            def emit_acts():
                nonlocal at
                at = apool.tile([P, kp, 2, b_shard], f8, tag="act")
                if no_in_dma:
                    nc.any.memset(at[:], 0)
                else:
                    nc.sync.dma_start(at[:], acts_d[:])

            fixed_wc = None
            if no_in_dma:
                fixed_wc = wpool.tile([P, MG_W, kp, 2 * P], f8, tag="wfix")
                nc.any.memset(fixed_wc[:], 0)
                emit_acts()

            with (tc.For_i(0, t_loop) if t_loop is not None else _null_ctx()):
                if not no_in_dma:
                    emit_acts()

                wc = ht = og = None
                for m in range(mt):
                    if no_in_dma:
                        wc = fixed_wc
                    elif m % MG_W == 0:
                        wc = wpool.tile([P, MG_W, kp, 2 * P], f8, tag="w")
                        nc.sync.dma_start(wc[:], wts_d[m // MG_W])
                    if not no_epilogue and m % MG_H == 0:
                        ht = hpool.tile([P, MG_H, b_shard], bf16, tag="h")
                        nc.sync.dma_start(ht[:], h16_d[m // MG_H])
                        og = opool.tile([P, MG_H, b_shard], bf16, tag="o")

                    psums = [pspool.tile([P, n_tile], f32, tag="ps",
                                         name=f"ps{m}_{n}") for n in range(nt)]
                    # all k-accumulation for one PSUM bank back-to-back
                    for n in range(nt):
                        for k in range(kp):
                            nc.tensor.matmul(
                                psums[n][:],
                                wc[:, 0 if no_in_dma else m % MG_W, k],
                                at[:, k, :, n * n_tile:(n + 1) * n_tile],
                                start=(k == 0),
                                stop=(k == kp - 1),
                                perf_mode=DR,
                            )
                        if not no_epilogue:
                            sl = slice(n * n_tile, (n + 1) * n_tile)
                            t = tpool.tile([P, n_tile], f32, tag="t")
                            nc.scalar.activation(t[:], psums[n][:], Tanh,
                                                 scale=1.0 / W_SCALE)
                            nc.vector.tensor_scalar_mul(
                                og[:, m % MG_H, sl], ht[:, m % MG_H, sl],
                                1.0 - LEAK)
                            nc.vector.scalar_tensor_tensor(
                                og[:, m % MG_H, sl], t[:], LEAK,
                                og[:, m % MG_H, sl],
                                mybir.AluOpType.mult, mybir.AluOpType.add,
                            )
                    if not no_epilogue and m % MG_H == MG_H - 1:
                        nc.sync.dma_start(out_d[m // MG_H], og[:])

    nc.compile()
    meta = dict(in_dim=in_dim, r_dim=r_dim, b_shard=b_shard,
                n_tile=n_tile, kp=kp, mt=mt, nt=nt)
    return nc, meta


def pack_weights(input_weights, reservoir_weights):
    """[R, IN] + [R, R] fp32 -> [mt/MG_W, P, MG_W, kp, 2P] fp8 in the
    DoubleRowSwInterleave layout (per partition: A127 B127 ... A0 B0)."""
    w = np.concatenate(
        [np.ascontiguousarray(input_weights.T), np.ascontiguousarray(reservoir_weights.T)],
        axis=0,
    )  # [in+r, r]: w[k, r]
    w8 = (w * W_SCALE).astype(F8NP)
    k_dim, r_dim = w8.shape
    kp, mt = k_dim // (2 * P), r_dim // P
    # w8[k, r]: k = 256*pair + 128*j + p, r = 128*m + c -> [m, pair, p, j, c]
    t = w8.reshape(kp, 2, P, mt, P).transpose(3, 0, 2, 1, 4)  # [mt, kp, P, 2, P]
    # SwInterleave: planes interleaved per column, columns reversed
    t = np.ascontiguousarray(t[..., ::-1].transpose(0, 1, 2, 4, 3))
    t = t.reshape(mt, kp, P, 2 * P).transpose(0, 2, 1, 3)     # [mt, P, kp, 2P]
    t = t.reshape(mt // MG_W, MG_W, P, kp, 2 * P).transpose(0, 2, 1, 3, 4)
    return np.ascontiguousarray(t)


def pack_acts(x_shard, h_shard):
    """[b, in] + [b, r] fp32 -> [P, kp, 2, b] fp8 (transposed, DoubleRow
    pair-tiled: plane j at partition p covers k = 256*pair + 128*j + p)."""
    a = np.concatenate([x_shard.T, h_shard.T], axis=0)  # [in+r, b]
    a8 = a.astype(F8NP)
    k_dim, b = a8.shape
    t = a8.reshape(k_dim // (2 * P), 2, P, b).transpose(2, 0, 1, 3)
    return np.ascontiguousarray(t)


def pack_h16(h_shard):
    """[b, r] fp32 -> [mt/MG_H, P, MG_H, b] bf16 tiles for the epilogue."""
    b, r = h_shard.shape
    t = h_shard.T.astype(BF16NP).reshape(r // (MG_H * P), MG_H, P, b)
    return np.ascontiguousarray(t.transpose(0, 2, 1, 3))


_CACHE = {}


def make_in_maps(inputs, prev_state, input_weights, reservoir_weights):
    x = np.ascontiguousarray(np.asarray(inputs, dtype=np.float32))
    h = np.ascontiguousarray(np.asarray(prev_state, dtype=np.float32))
    wi = np.asarray(input_weights, dtype=np.float32)
    wr = np.asarray(reservoir_weights, dtype=np.float32)
    assert x.shape == (B_FULL, IN_DIM) and h.shape == (B_FULL, R_DIM)

    wts = pack_weights(wi, wr)
    in_maps = []
    for c in range(N_CORES):
        sl = slice(c * B_SHARD, (c + 1) * B_SHARD)
        in_maps.append({
            "acts": pack_acts(x[sl], h[sl]),
            "wts": wts,
            "h16": pack_h16(h[sl]),
        })
    return in_maps


def kernel(inputs, prev_state, input_weights, reservoir_weights):
    from concourse import bass_utils

    if "nc" not in _CACHE:
        _CACHE["nc"], _CACHE["meta"] = build_program()
    nc = _CACHE["nc"]

    in_maps = make_in_maps(inputs, prev_state, input_weights, reservoir_weights)
    res = bass_utils.run_bass_kernel_spmd(nc, in_maps, core_ids=list(range(N_CORES)))

    out = np.empty((B_FULL, R_DIM), dtype=np.float32)
    for c in range(N_CORES):
        o = res.results[c]["out"]  # [mt/MG_H, P, MG_H, b_shard] bf16
        o = o.transpose(0, 2, 1, 3).reshape(R_DIM, B_SHARD)
        out[c * B_SHARD:(c + 1) * B_SHARD] = o.T.astype(np.float32)
    return out
